# revision 14
# baseline (speedup 1.0000x reference)
"""ConvGAU (gated attention unit with 1x1 conv projections) on 8 TRN2 NeuronCores.

Data-parallel: B=16 images sharded 2-per-core across 8 cores; every op is
batch-independent so there is no cross-core communication.

Per-image compute (C=256, N=48*48=2304, HID=512, QK=96):

  q,k  = silu(w_qk x + b_qk)   [96, N]  bf16
  vT   = silu(x^T w_v^T + b_v) [N, 512] fp8e4m3, paired [128,2,512] (j=2t,2t+1)
  gate = silu(w_g x + b_g)     [512, N] bf16
  per n-chunk (<=512 cols):
    simT_j = k_j^T q_chunk     [128, S] PSUM, bf16 matmul (j = 18 pos chunks)
    rl_j   = relu(simT_j)      DVE tensor_scalar_max or ACT Relu (RELU_ACT)
    AT_j   = rl^2 -> fp8       Pool tensor_tensor (SQ_POOL pairs) or ACT Square
    V[hs] += vT_pair^T@AT_pair fp8 DoubleRow (2 k-tiles/instr, 0.5 cyc/row),
                               PSUM-accumulated over the 9 pairs
    Vg[hs] = V[hs]*gate        DVE (f32 psum x bf16 -> f32r)
    out[os] = sum_hs w_oT^T @ Vg[hs]; (out + b_out) + x; DMA out

The emission is software-pipelined ACROSS IMAGES: image i's attention phase
(the elementwise-heavy part: relu/square on DVE/ACT/Pool) is interleaved at
pair granularity with image i+1's entire projection phase (PE matmuls + ACT
silu), because attention needs all of its own image's k/vT/gate first. This
keeps all four engines near their ~62-68us/img busy floor.
"""

import numpy as np
from contextlib import ExitStack

import concourse.bass as bass
import concourse.tile as tile
from concourse import bacc
from concourse import mybir
from concourse.bass_utils import run_bass_kernel_spmd

B, C, N = 16, 256, 48 * 48
HID, QK = 512, 96
NCORES = 8
BPC = B // NCORES  # images per core

F32 = mybir.dt.float32
F32R = mybir.dt.float32r
F8 = mybir.dt.float8e4
BF16 = mybir.dt.bfloat16
AF = mybir.ActivationFunctionType
ALU = mybir.AluOpType
DR = mybir.MatmulPerfMode.DoubleRow

# n-chunks of the 2304 spatial positions (free dim of most matmuls; <=512 fits
# one PSUM bank)
NCH = [(0, 512), (512, 512), (1024, 512), (1536, 512), (2048, 256)]
NJ = N // 128  # 18 key/position chunks
NP = NJ // 2   # 9 pairs (fp8 DoubleRow processes 2 k-tiles per matmul)
LAG = 2        # V consumption lags AT production by this many pairs

# relu^2 -> fp8 runs as two ops (the walrus verifier rejects one DVE op
# reading the same PSUM tensor twice, so the fused (x max 0)*x form is not
# encodable). Engine assignment balances measured per-op costs
# (DVE relu 756ns, ACT relu/silu 612ns, ACT square 578ns, Pool square 999ns)
# to ~66us/img on each of ACT/DVE/Pool (PE is 61.4us/img).
SQ_POOL = {0, 1, 2, 3, 4, 5}   # pairs whose squares go to gpsimd
RELU_ACT = {2, 5, 8}           # pairs whose relus go to ACT (else DVE)


def _r(ap):
    return ap.bitcast(F32R)


def build_bass(with_bv: bool, act=None, reps: int = 1) -> bass.Bass:
    if act is None:
        act = AF.Silu
    nc = bacc.Bacc("TRN2", target_bir_lowering=False, debug=False)

    # packed weights (host-side prep): w_qk [C, 192] = [w_qT | w_kT];
    # w_vg [C, 1024] = [w_vT | w_gT]; w_op [128, 1024] = 4 chunks of w_oT
    # side by side; b_pack [128, 8] = [b_q, b_k, b_g0..3, b_o0..1] columns.
    x_d = nc.dram_tensor("x", [BPC, C, N], F32, kind="ExternalInput").ap()
    wqk_d = nc.dram_tensor("w_qk", [C, 2 * QK], F32, kind="ExternalInput").ap()
    wvg_d = nc.dram_tensor("w_vg", [C, 2 * HID], F32, kind="ExternalInput").ap()
    wop_d = nc.dram_tensor("w_op", [128, 4 * C], F32, kind="ExternalInput").ap()
    bp_d = nc.dram_tensor("b_pack", [128, 8], F32, kind="ExternalInput").ap()
    bv_d = None
    if with_bv:
        bv_d = nc.dram_tensor("b_v_bc", [128, HID], F32, kind="ExternalInput").ap()
    out_d = nc.dram_tensor("out", [BPC, C, N], F32, kind="ExternalOutput").ap()

    with tile.TileContext(nc) as tc, ExitStack() as ctx:
        consts = ctx.enter_context(tc.tile_pool(name="consts", bufs=1))
        xp = ctx.enter_context(tc.tile_pool(name="xp", bufs=4))
        qkp = ctx.enter_context(tc.tile_pool(name="qkp", bufs=4))
        gp = ctx.enter_context(tc.tile_pool(name="gp", bufs=8))
        vtp = ctx.enter_context(tc.tile_pool(name="vtp", bufs=2 * NP))
        atp = ctx.enter_context(tc.tile_pool(name="atp", bufs=10))
        rlp = ctx.enter_context(tc.tile_pool(name="rlp", bufs=14))
        vgp = ctx.enter_context(tc.tile_pool(name="vgp", bufs=8))
        xrp = ctx.enter_context(tc.tile_pool(name="xrp", bufs=4))
        obp = ctx.enter_context(tc.tile_pool(name="obp", bufs=4))
        simp = ctx.enter_context(tc.tile_pool(name="simp", bufs=3, space="PSUM"))
        projp = ctx.enter_context(tc.tile_pool(name="projp", bufs=1, space="PSUM"))
        vpsp = ctx.enter_context(tc.tile_pool(name="vpsp", bufs=4, space="PSUM"))

        # ---- one-time packed weight loads on the SWDGE queue ----
        wqk_sb = [consts.tile([128, 2 * QK], F32R, name=f"wqk{c}", tag=f"wqk{c}")
                  for c in range(2)]
        for c in range(2):
            nc.gpsimd.dma_start(wqk_sb[c][:], _r(wqk_d[c * 128:(c + 1) * 128, :]))
        bp_sb = consts.tile([128, 8], F32, name="bp", tag="bp")
        nc.gpsimd.dma_start(bp_sb[:], bp_d[:, :])
        wvg_sb = [consts.tile([128, 2 * HID], F32R, name=f"wvg{c}", tag=f"wvg{c}")
                  for c in range(2)]
        for c in range(2):
            nc.gpsimd.dma_start(wvg_sb[c][:, 0:HID], _r(wvg_d[c * 128:(c + 1) * 128, 0:HID]))
        for c in range(2):
            nc.gpsimd.dma_start(wvg_sb[c][:, HID:2 * HID],
                                _r(wvg_d[c * 128:(c + 1) * 128, HID:2 * HID]))
        wop_sb = consts.tile([128, 4 * C], F32R, name="wop", tag="wop")
        nc.gpsimd.dma_start(wop_sb[:], _r(wop_d[:, :]))

        wq_sb = [wqk_sb[c][:, 0:QK] for c in range(2)]
        wk_sb = [wqk_sb[c][:, QK:2 * QK] for c in range(2)]
        wv_sb = [wvg_sb[c][:, 0:HID] for c in range(2)]
        wg_sb = [wvg_sb[c][:, HID:2 * HID] for c in range(2)]
        wo_sb = [wop_sb[:, h * C:(h + 1) * C] for h in range(4)]
        bq_sb = bp_sb[0:QK, 0:1]
        bk_sb = bp_sb[0:QK, 1:2]
        bg_sb = [bp_sb[:, 2 + h:3 + h] for h in range(4)]
        bo_sb = [bp_sb[:, 6 + o:7 + o] for o in range(2)]
        bv_sb = None
        if with_bv:
            bv_sb = consts.tile([128, HID], F32, name="bv", tag="bv")
            nc.sync.dma_start(bv_sb[:], bv_d[:, :])

        def make_state(seq):
            """Allocate image tiles, emit x DMAs, build projection thunks."""
            img = seq % BPC
            st = {}
            st["x"] = [xp.tile([128, N], F32R, name=f"x{seq}_{c}", tag="x")
                       for c in range(2)]
            for (n0, S) in NCH:
                for c in range(2):
                    nc.sync.dma_start(st["x"][c][:, n0:n0 + S],
                                      _r(x_d[img, c * 128:(c + 1) * 128, n0:n0 + S]))
            st["q"] = qkp.tile([QK, N], BF16, name=f"q{seq}", tag="qk")
            st["k"] = qkp.tile([QK, N], BF16, name=f"k{seq}", tag="qk")
            st["g"] = [gp.tile([128, N], BF16, name=f"g{seq}_{hs}", tag="g")
                       for hs in range(4)]
            st["vt"] = [vtp.tile([128, 2, HID], F8, name=f"vt{seq}_{t}", tag="vt")
                        for t in range(NP)]
            st["img"] = img

            thunks = []
            x_sb = st["x"]
            for (n0, S) in NCH:
                def qk_thunk(n0=n0, S=S, st=st):
                    for dst, w_sb, b_sb in ((st["q"], wq_sb, bq_sb),
                                            (st["k"], wk_sb, bk_sb)):
                        ps = projp.tile([QK, 512], F32, name="ps_qk", tag="pj")
                        nc.tensor.matmul(ps[:, :S], w_sb[0][:], x_sb[0][:, n0:n0 + S],
                                         start=True, stop=False)
                        nc.tensor.matmul(ps[:, :S], w_sb[1][:], x_sb[1][:, n0:n0 + S],
                                         start=False, stop=True)
                        nc.scalar.activation(dst[:, n0:n0 + S], ps[:, :S], act,
                                             bias=b_sb)
                thunks.append(qk_thunk)
                for j in range(n0 // 128, (n0 + S) // 128):
                    def vt_thunk(j=j, st=st):
                        ps = projp.tile([128, 512], F32, name="ps_v", tag="pj")
                        nc.tensor.matmul(ps[:], x_sb[0][:, j * 128:(j + 1) * 128],
                                         wv_sb[0][:], start=True, stop=False)
                        nc.tensor.matmul(ps[:], x_sb[1][:, j * 128:(j + 1) * 128],
                                         wv_sb[1][:], start=False, stop=True)
                        if with_bv:
                            nc.vector.tensor_add(ps[:], ps[:], bv_sb[:])
                        nc.scalar.activation(st["vt"][j // 2][:, j % 2, :], ps[:], act)
                    thunks.append(vt_thunk)
                for hs in range(4):
                    def g_thunk(n0=n0, S=S, hs=hs, st=st):
                        ps = projp.tile([128, 512], F32, name="ps_g", tag="pj")
                        nc.tensor.matmul(ps[:, :S], wg_sb[0][:, hs * 128:(hs + 1) * 128],
                                         x_sb[0][:, n0:n0 + S], start=True, stop=False)
                        nc.tensor.matmul(ps[:, :S], wg_sb[1][:, hs * 128:(hs + 1) * 128],
                                         x_sb[1][:, n0:n0 + S], start=False, stop=True)
                        nc.scalar.activation(st["g"][hs][:, n0:n0 + S], ps[:, :S], act,
                                             bias=bg_sb[hs])
                    thunks.append(g_thunk)
            st["thunks"] = thunks
            st["ti"] = 0
            return st

        def pop_thunk(st, n=1):
            if st is None:
                return
            for _ in range(n):
                if st["ti"] < len(st["thunks"]):
                    st["thunks"][st["ti"]]()
                    st["ti"] += 1

        def flush_thunks(st):
            if st is not None:
                pop_thunk(st, len(st["thunks"]) - st["ti"])

        def emit_outproj(st, n0, S, vg):
            img = st["img"]
            for os in range(2):
                ps = projp.tile([128, 512], F32, name="ps_o", tag="pj")
                for hs in range(4):
                    nc.tensor.matmul(ps[:, :S],
                                     wo_sb[hs][:, os * 128:(os + 1) * 128],
                                     vg[hs][:, :S],
                                     start=(hs == 0), stop=(hs == 3),
                                     skip_group_check=True)
                xr = xrp.tile([128, 512], F32, name="xr", tag="xr")
                nc.sync.dma_start(xr[:, :S], x_d[img, os * 128:(os + 1) * 128,
                                                 n0:n0 + S])
                ob = obp.tile([128, 512], F32, name="ob", tag="ob")
                # (psum + b_out) + x_residual
                nc.vector.scalar_tensor_tensor(ob[:, :S], ps[:, :S],
                                               bo_sb[os], xr[:, :S],
                                               ALU.add, ALU.add)
                nc.sync.dma_start(out_d[img, os * 128:(os + 1) * 128, n0:n0 + S],
                                  ob[:, :S])

        def emit_v_pair(st, vps, tp, S):
            for hs in range(4):
                nc.tensor.matmul(
                    vps[hs][:, :S],
                    st["vt"][tp][:, :, hs * 128:(hs + 1) * 128],
                    st["at"][tp][:, :, :S],
                    start=(tp == 0), stop=(tp == NP - 1),
                    perf_mode=DR, skip_group_check=True)

        def attention(st, nxt):
            pending = None
            q_sb, k_sb = st["q"], st["k"]
            for (n0, S) in NCH:
                vps = [vpsp.tile([128, 512], F32, name=f"vps{hs}", tag="vps")
                       for hs in range(4)]
                st["at"] = [None] * NP
                for t in range(NP):
                    at = atp.tile([128, 2, 512], F8, name="at", tag="at")
                    st["at"][t] = at
                    for half in range(2):
                        j = 2 * t + half
                        sim = simp.tile([128, 512], F32, name="ps_sim", tag="sim")
                        nc.tensor.matmul(sim[:, :S], k_sb[:, j * 128:(j + 1) * 128],
                                         q_sb[:, n0:n0 + S], start=True, stop=True)
                        rl = rlp.tile([128, 512], BF16, name="rl", tag="rl")
                        if t in RELU_ACT:
                            nc.scalar.activation(rl[:, :S], sim[:, :S], AF.Relu)
                        else:
                            nc.vector.tensor_scalar_max(rl[:, :S], sim[:, :S], 0.0)
                        if t in SQ_POOL:
                            nc.gpsimd.tensor_tensor(at[:, half, :S], rl[:, :S],
                                                    rl[:, :S], ALU.mult)
                        else:
                            nc.scalar.square(at[:, half, :S], rl[:, :S])
                        if j == 1 and pending is not None:
                            emit_outproj(st, *pending)
                            pending = None
                    pop_thunk(nxt)
                    if t >= LAG:
                        emit_v_pair(st, vps, t - LAG, S)
                for tp in range(NP - LAG, NP):
                    emit_v_pair(st, vps, tp, S)
                pop_thunk(nxt)

                vg = []
                for hs in range(4):
                    vgt = vgp.tile([128, 512], F32R, name="vg", tag="vg")
                    nc.vector.tensor_mul(vgt[:, :S], vps[hs][:, :S],
                                         st["g"][hs][:, n0:n0 + S])
                    vg.append(vgt)
                pending = (n0, S, vg)
            emit_outproj(st, *pending)

        seq_len = reps * BPC
        cur = make_state(0)
        flush_thunks(cur)  # prologue: image 0's projections up front
        for i in range(seq_len):
            nxt = make_state(i + 1) if i + 1 < seq_len else None
            attention(cur, nxt)
            flush_thunks(nxt)
            cur = nxt
    nc.compile()
    return nc


_CACHE = {}


def _get_nc(with_bv: bool) -> bass.Bass:
    if with_bv not in _CACHE:
        _CACHE[with_bv] = build_bass(with_bv)
    return _CACHE[with_bv]


def _make_in_maps(inputs: dict):
    x = np.ascontiguousarray(np.asarray(inputs["x"], dtype=np.float32))
    w_hidden = np.asarray(inputs["w_hidden"], dtype=np.float32)
    b_hidden = np.asarray(inputs["b_hidden"], dtype=np.float32)
    w_qk = np.asarray(inputs["w_qk"], dtype=np.float32)
    b_qk = np.asarray(inputs["b_qk"], dtype=np.float32)
    w_out = np.asarray(inputs["w_out"], dtype=np.float32)
    b_out = np.asarray(inputs["b_out"], dtype=np.float32)

    b_v = b_hidden[:HID]
    with_bv = bool(np.any(b_v != 0.0))

    w_oT = w_out.T  # [HID, C]
    w_op = np.ascontiguousarray(
        w_oT.reshape(4, 128, C).transpose(1, 0, 2).reshape(128, 4 * C))
    b_pack = np.zeros((128, 8), np.float32)
    b_pack[:QK, 0] = b_qk[:QK]
    b_pack[:QK, 1] = b_qk[QK:]
    b_pack[:, 2:6] = b_hidden[HID:].reshape(4, 128).T
    b_pack[:, 6:8] = b_out.reshape(2, 128).T
    base = {
        "w_qk": np.ascontiguousarray(w_qk.T),
        "w_vg": np.ascontiguousarray(w_hidden.T),
        "w_op": w_op,
        "b_pack": b_pack,
    }
    if with_bv:
        base["b_v_bc"] = np.ascontiguousarray(np.tile(b_v[None, :], (128, 1)))

    xs = x.reshape(B, C, N)
    in_maps = [
        {**base, "x": np.ascontiguousarray(xs[i * BPC:(i + 1) * BPC])}
        for i in range(NCORES)
    ]
    return in_maps, with_bv


def _run(inputs: dict, trace: bool = False):
    in_maps, with_bv = _make_in_maps(inputs)
    nc = _get_nc(with_bv)
    res = run_bass_kernel_spmd(nc, in_maps, core_ids=list(range(NCORES)),
                               trace=trace)
    out = np.concatenate([res.results[i]["out"] for i in range(NCORES)], axis=0)
    return out.reshape(B, C, 48, 48), res


def kernel(**inputs) -> np.ndarray:
    out, _ = _run(inputs, trace=False)
    return out


# revision 15
# speedup vs baseline: 1.2793x; 1.2793x over previous
"""ConvGAU (gated attention unit with 1x1 conv projections) on 8 TRN2 NeuronCores.

Data-parallel: B=16 images sharded 2-per-core across 8 cores; every op is
batch-independent so there is no cross-core communication.

Per-image compute (C=256, N=48*48=2304, HID=512, QK=96):

  q,k  = silu(w_qk x + b_qk)   [96, N]  bf16
  vT   = silu(x^T w_v^T + b_v) [N, 512] fp8e4m3, paired [128,2,512] (j=2t,2t+1)
  gate = silu(w_g x + b_g)     [512, N] bf16
  per n-chunk (<=512 cols):
    simT_j = k_j^T q_chunk     [128, S] PSUM, bf16 matmul (j = 18 pos chunks)
    rl_j   = relu(simT_j)      DVE tensor_scalar_max or ACT Relu (RELU_ACT)
    AT_j   = rl^2 -> fp8       Pool tensor_tensor (SQ_POOL pairs) or ACT Square
    V[hs] += vT_pair^T@AT_pair fp8 DoubleRow (2 k-tiles/instr, 0.5 cyc/row),
                               PSUM-accumulated over the 9 pairs
    Vg[hs] = V[hs]*gate        DVE (f32 psum x bf16 -> f32r)
    out[os] = sum_hs w_oT^T @ Vg[hs]; (out + b_out) + x; DMA out

The emission is software-pipelined ACROSS IMAGES: image i's attention phase
(the elementwise-heavy part: relu/square on DVE/ACT/Pool) is interleaved at
pair granularity with image i+1's entire projection phase (PE matmuls + ACT
silu), because attention needs all of its own image's k/vT/gate first. This
keeps all four engines near their ~62-68us/img busy floor.
"""

import numpy as np
from contextlib import ExitStack

import concourse.bass as bass
import concourse.tile as tile
from concourse import bacc
from concourse import mybir
from concourse.bass_utils import run_bass_kernel_spmd

B, C, N = 16, 256, 48 * 48
HID, QK = 512, 96
NCORES = 8
BPC = B // NCORES  # images per core

F32 = mybir.dt.float32
F32R = mybir.dt.float32r
F8 = mybir.dt.float8e4
BF16 = mybir.dt.bfloat16
AF = mybir.ActivationFunctionType
ALU = mybir.AluOpType
DR = mybir.MatmulPerfMode.DoubleRow

# n-chunks of the 2304 spatial positions (free dim of most matmuls; <=512 fits
# one PSUM bank)
NCH = [(0, 512), (512, 512), (1024, 512), (1536, 512), (2048, 256)]
NJ = N // 128  # 18 key/position chunks
NP = NJ // 2   # 9 pairs (fp8 DoubleRow processes 2 k-tiles per matmul)
LAG = 2        # V consumption lags AT production by this many pairs

# relu^2 -> fp8 runs as two ops (the walrus verifier rejects one DVE op
# reading the same PSUM tensor twice, so the fused (x max 0)*x form is not
# encodable). Engine assignment balances measured per-op costs
# (DVE relu 756ns, ACT relu/silu 612ns, ACT square 578ns, Pool square 999ns)
# to ~66us/img on each of ACT/DVE/Pool (PE is 61.4us/img).
# fractions of the 90 units/img: squares 63/90 on Pool (else ACT), relus
# 33/90 on ACT (else DVE) -- spread evenly via fractional accumulators
SQ_POOL_FRAC = 63.0 / 90.0
RELU_ACT_FRAC = 33.0 / 90.0


def _r(ap):
    return ap.bitcast(F32R)


def build_bass(with_bv: bool, act=None, reps: int = 1) -> bass.Bass:
    if act is None:
        act = AF.Silu
    nc = bacc.Bacc("TRN2", target_bir_lowering=False, debug=False)

    # packed weights (host-side prep): w_qk [C, 192] = [w_qT | w_kT];
    # w_vg [C, 1024] = [w_vT | w_gT]; w_op [128, 1024] = 4 chunks of w_oT
    # side by side; b_pack [128, 8] = [b_q, b_k, b_g0..3, b_o0..1] columns.
    x_d = nc.dram_tensor("x", [BPC, C, N], F32, kind="ExternalInput").ap()
    wqk_d = nc.dram_tensor("w_qk", [C, 2 * QK], F32, kind="ExternalInput").ap()
    wvg_d = nc.dram_tensor("w_vg", [C, 2 * HID], F32, kind="ExternalInput").ap()
    wop_d = nc.dram_tensor("w_op", [128, 4 * C], F32, kind="ExternalInput").ap()
    bp_d = nc.dram_tensor("b_pack", [128, 8], F32, kind="ExternalInput").ap()
    bv_d = None
    if with_bv:
        bv_d = nc.dram_tensor("b_v_bc", [128, HID], F32, kind="ExternalInput").ap()
    out_d = nc.dram_tensor("out", [BPC, C, N], F32, kind="ExternalOutput").ap()

    with tile.TileContext(nc) as tc, ExitStack() as ctx:
        consts = ctx.enter_context(tc.tile_pool(name="consts", bufs=1))
        xp = ctx.enter_context(tc.tile_pool(name="xp", bufs=4))
        qkp = ctx.enter_context(tc.tile_pool(name="qkp", bufs=4))
        gp = ctx.enter_context(tc.tile_pool(name="gp", bufs=8))
        vtp = ctx.enter_context(tc.tile_pool(name="vtp", bufs=2 * NP))
        atp = ctx.enter_context(tc.tile_pool(name="atp", bufs=10))
        rlp = ctx.enter_context(tc.tile_pool(name="rlp", bufs=14))
        vgp = ctx.enter_context(tc.tile_pool(name="vgp", bufs=8))
        xrp = ctx.enter_context(tc.tile_pool(name="xrp", bufs=4))
        obp = ctx.enter_context(tc.tile_pool(name="obp", bufs=4))
        simp = ctx.enter_context(tc.tile_pool(name="simp", bufs=2, space="PSUM"))
        projp = ctx.enter_context(tc.tile_pool(name="projp", bufs=2, space="PSUM"))
        vpsp = ctx.enter_context(tc.tile_pool(name="vpsp", bufs=4, space="PSUM"))

        # ---- one-time packed weight loads on the SWDGE queue ----
        wqk_sb = [consts.tile([128, 2 * QK], F32R, name=f"wqk{c}", tag=f"wqk{c}")
                  for c in range(2)]
        for c in range(2):
            nc.gpsimd.dma_start(wqk_sb[c][:], _r(wqk_d[c * 128:(c + 1) * 128, :]))
        bp_sb = consts.tile([128, 8], F32, name="bp", tag="bp")
        nc.gpsimd.dma_start(bp_sb[:], bp_d[:, :])
        wvg_sb = [consts.tile([128, 2 * HID], F32R, name=f"wvg{c}", tag=f"wvg{c}")
                  for c in range(2)]
        for c in range(2):
            nc.gpsimd.dma_start(wvg_sb[c][:, 0:HID], _r(wvg_d[c * 128:(c + 1) * 128, 0:HID]))
        for c in range(2):
            nc.gpsimd.dma_start(wvg_sb[c][:, HID:2 * HID],
                                _r(wvg_d[c * 128:(c + 1) * 128, HID:2 * HID]))
        wop_sb = consts.tile([128, 4 * C], F32R, name="wop", tag="wop")
        nc.gpsimd.dma_start(wop_sb[:], _r(wop_d[:, :]))

        wq_sb = [wqk_sb[c][:, 0:QK] for c in range(2)]
        wk_sb = [wqk_sb[c][:, QK:2 * QK] for c in range(2)]
        wv_sb = [wvg_sb[c][:, 0:HID] for c in range(2)]
        wg_sb = [wvg_sb[c][:, HID:2 * HID] for c in range(2)]
        wo_sb = [wop_sb[:, h * C:(h + 1) * C] for h in range(4)]
        bq_sb = bp_sb[0:QK, 0:1]
        bk_sb = bp_sb[0:QK, 1:2]
        bg_sb = [bp_sb[:, 2 + h:3 + h] for h in range(4)]
        bo_sb = [bp_sb[:, 6 + o:7 + o] for o in range(2)]
        bv_sb = None
        if with_bv:
            bv_sb = consts.tile([128, HID], F32, name="bv", tag="bv")
            nc.sync.dma_start(bv_sb[:], bv_d[:, :])

        def make_state(seq):
            """Allocate image tiles, emit x DMAs, build projection thunks."""
            img = seq % BPC
            st = {}
            st["x"] = [xp.tile([128, N], F32R, name=f"x{seq}_{c}", tag="x")
                       for c in range(2)]
            for (n0, S) in NCH:
                for c in range(2):
                    nc.sync.dma_start(st["x"][c][:, n0:n0 + S],
                                      _r(x_d[img, c * 128:(c + 1) * 128, n0:n0 + S]))
            st["q"] = qkp.tile([QK, N], BF16, name=f"q{seq}", tag="qk")
            st["k"] = qkp.tile([QK, N], BF16, name=f"k{seq}", tag="qk")
            st["g"] = [gp.tile([128, N], BF16, name=f"g{seq}_{hs}", tag="g")
                       for hs in range(4)]
            st["vt"] = [vtp.tile([128, 2, HID], F8, name=f"vt{seq}_{t}", tag="vt")
                        for t in range(NP)]
            st["img"] = img

            thunks = []
            x_sb = st["x"]
            for (n0, S) in NCH:
                def qk_thunk(n0=n0, S=S, st=st):
                    for dst, w_sb, b_sb in ((st["q"], wq_sb, bq_sb),
                                            (st["k"], wk_sb, bk_sb)):
                        ps = projp.tile([QK, 512], F32, name="ps_qk", tag="pj")
                        nc.tensor.matmul(ps[:, :S], w_sb[0][:], x_sb[0][:, n0:n0 + S],
                                         start=True, stop=False)
                        nc.tensor.matmul(ps[:, :S], w_sb[1][:], x_sb[1][:, n0:n0 + S],
                                         start=False, stop=True)
                        nc.scalar.activation(dst[:, n0:n0 + S], ps[:, :S], act,
                                             bias=b_sb)
                thunks.append(qk_thunk)
                for j in range(n0 // 128, (n0 + S) // 128):
                    def vt_thunk(j=j, st=st):
                        ps = projp.tile([128, 512], F32, name="ps_v", tag="pj")
                        nc.tensor.matmul(ps[:], x_sb[0][:, j * 128:(j + 1) * 128],
                                         wv_sb[0][:], start=True, stop=False)
                        nc.tensor.matmul(ps[:], x_sb[1][:, j * 128:(j + 1) * 128],
                                         wv_sb[1][:], start=False, stop=True)
                        if with_bv:
                            nc.vector.tensor_add(ps[:], ps[:], bv_sb[:])
                        nc.scalar.activation(st["vt"][j // 2][:, j % 2, :], ps[:], act)
                    thunks.append(vt_thunk)
                for hs in range(4):
                    def g_thunk(n0=n0, S=S, hs=hs, st=st):
                        ps = projp.tile([128, 512], F32, name="ps_g", tag="pj")
                        nc.tensor.matmul(ps[:, :S], wg_sb[0][:, hs * 128:(hs + 1) * 128],
                                         x_sb[0][:, n0:n0 + S], start=True, stop=False)
                        nc.tensor.matmul(ps[:, :S], wg_sb[1][:, hs * 128:(hs + 1) * 128],
                                         x_sb[1][:, n0:n0 + S], start=False, stop=True)
                        nc.scalar.activation(st["g"][hs][:, n0:n0 + S], ps[:, :S], act,
                                             bias=bg_sb[hs])
                    thunks.append(g_thunk)
            st["thunks"] = thunks
            st["ti"] = 0
            return st

        def pop_thunk(st, n=1):
            if st is None:
                return
            for _ in range(n):
                if st["ti"] < len(st["thunks"]):
                    st["thunks"][st["ti"]]()
                    st["ti"] += 1

        def flush_thunks(st):
            if st is not None:
                pop_thunk(st, len(st["thunks"]) - st["ti"])

        def emit_outproj(st, n0, S, vg):
            img = st["img"]
            for os in range(2):
                ps = projp.tile([128, 512], F32, name="ps_o", tag="pj")
                for hs in range(4):
                    nc.tensor.matmul(ps[:, :S],
                                     wo_sb[hs][:, os * 128:(os + 1) * 128],
                                     vg[hs][:, :S],
                                     start=(hs == 0), stop=(hs == 3),
                                     skip_group_check=True)
                xr = xrp.tile([128, 512], F32, name="xr", tag="xr")
                nc.sync.dma_start(xr[:, :S], x_d[img, os * 128:(os + 1) * 128,
                                                 n0:n0 + S])
                ob = obp.tile([128, 512], F32, name="ob", tag="ob")
                # (psum + b_out) + x_residual
                nc.vector.scalar_tensor_tensor(ob[:, :S], ps[:, :S],
                                               bo_sb[os], xr[:, :S],
                                               ALU.add, ALU.add)
                nc.sync.dma_start(out_d[img, os * 128:(os + 1) * 128, n0:n0 + S],
                                  ob[:, :S])

        def emit_v_pair(st, vps, tp, S):
            for hs in range(4):
                nc.tensor.matmul(
                    vps[hs][:, :S],
                    st["vt"][tp][:, :, hs * 128:(hs + 1) * 128],
                    st["at"][tp][:, :, :S],
                    start=(tp == 0), stop=(tp == NP - 1),
                    perf_mode=DR, skip_group_check=True)

        def attention(st, nxt):
            pending = None
            st["racc"] = 0.0
            st["sacc"] = 0.0
            q_sb, k_sb = st["q"], st["k"]
            for (n0, S) in NCH:
                vps = [vpsp.tile([128, 512], F32, name=f"vps{hs}", tag="vps")
                       for hs in range(4)]
                st["at"] = [None] * NP
                for t in range(NP):
                    at = atp.tile([128, 2, 512], F8, name="at", tag="at")
                    st["at"][t] = at
                    for half in range(2):
                        j = 2 * t + half
                        sim = simp.tile([128, 512], F32, name="ps_sim", tag="sim")
                        nc.tensor.matmul(sim[:, :S], k_sb[:, j * 128:(j + 1) * 128],
                                         q_sb[:, n0:n0 + S], start=True, stop=True)
                        rl = rlp.tile([128, 512], BF16, name="rl", tag="rl")
                        st["racc"] += RELU_ACT_FRAC
                        if st["racc"] >= 1.0:
                            st["racc"] -= 1.0
                            nc.scalar.activation(rl[:, :S], sim[:, :S], AF.Relu)
                        else:
                            nc.vector.tensor_scalar_max(rl[:, :S], sim[:, :S], 0.0)
                        st["sacc"] += SQ_POOL_FRAC
                        if st["sacc"] >= 1.0:
                            st["sacc"] -= 1.0
                            nc.gpsimd.tensor_tensor(at[:, half, :S], rl[:, :S],
                                                    rl[:, :S], ALU.mult)
                        else:
                            nc.scalar.square(at[:, half, :S], rl[:, :S])
                        if j == 1 and pending is not None:
                            emit_outproj(st, *pending)
                            pending = None
                    pop_thunk(nxt)
                    if t >= LAG:
                        emit_v_pair(st, vps, t - LAG, S)
                for tp in range(NP - LAG, NP):
                    emit_v_pair(st, vps, tp, S)
                pop_thunk(nxt)

                vg = []
                for hs in range(4):
                    vgt = vgp.tile([128, 512], F32R, name="vg", tag="vg")
                    nc.vector.tensor_mul(vgt[:, :S], vps[hs][:, :S],
                                         st["g"][hs][:, n0:n0 + S])
                    vg.append(vgt)
                pending = (n0, S, vg)
            emit_outproj(st, *pending)

        seq_len = reps * BPC
        cur = make_state(0)
        flush_thunks(cur)  # prologue: image 0's projections up front
        for i in range(seq_len):
            nxt = make_state(i + 1) if i + 1 < seq_len else None
            attention(cur, nxt)
            flush_thunks(nxt)
            cur = nxt
    nc.compile()
    return nc


_CACHE = {}


def _get_nc(with_bv: bool) -> bass.Bass:
    if with_bv not in _CACHE:
        _CACHE[with_bv] = build_bass(with_bv)
    return _CACHE[with_bv]


def _make_in_maps(inputs: dict):
    x = np.ascontiguousarray(np.asarray(inputs["x"], dtype=np.float32))
    w_hidden = np.asarray(inputs["w_hidden"], dtype=np.float32)
    b_hidden = np.asarray(inputs["b_hidden"], dtype=np.float32)
    w_qk = np.asarray(inputs["w_qk"], dtype=np.float32)
    b_qk = np.asarray(inputs["b_qk"], dtype=np.float32)
    w_out = np.asarray(inputs["w_out"], dtype=np.float32)
    b_out = np.asarray(inputs["b_out"], dtype=np.float32)

    b_v = b_hidden[:HID]
    with_bv = bool(np.any(b_v != 0.0))

    w_oT = w_out.T  # [HID, C]
    w_op = np.ascontiguousarray(
        w_oT.reshape(4, 128, C).transpose(1, 0, 2).reshape(128, 4 * C))
    b_pack = np.zeros((128, 8), np.float32)
    b_pack[:QK, 0] = b_qk[:QK]
    b_pack[:QK, 1] = b_qk[QK:]
    b_pack[:, 2:6] = b_hidden[HID:].reshape(4, 128).T
    b_pack[:, 6:8] = b_out.reshape(2, 128).T
    base = {
        "w_qk": np.ascontiguousarray(w_qk.T),
        "w_vg": np.ascontiguousarray(w_hidden.T),
        "w_op": w_op,
        "b_pack": b_pack,
    }
    if with_bv:
        base["b_v_bc"] = np.ascontiguousarray(np.tile(b_v[None, :], (128, 1)))

    xs = x.reshape(B, C, N)
    in_maps = [
        {**base, "x": np.ascontiguousarray(xs[i * BPC:(i + 1) * BPC])}
        for i in range(NCORES)
    ]
    return in_maps, with_bv


def _run(inputs: dict, trace: bool = False):
    in_maps, with_bv = _make_in_maps(inputs)
    nc = _get_nc(with_bv)
    res = run_bass_kernel_spmd(nc, in_maps, core_ids=list(range(NCORES)),
                               trace=trace)
    out = np.concatenate([res.results[i]["out"] for i in range(NCORES)], axis=0)
    return out.reshape(B, C, 48, 48), res


def kernel(**inputs) -> np.ndarray:
    out, _ = _run(inputs, trace=False)
    return out
